# revision 1
# baseline (speedup 1.0000x reference)
"""Trainium2 Bass kernel for nn_Decoder_51539607552479.

DecoderModule.forward: bilinear-upsample xt (32->64, align_corners) ->
xfuse = xup + alpha*xm -> conv3x3(512->512)+BN+ReLU -> conv3x3(512->256)
+BN+ReLU.  Pure data parallel: batch dim (8) across the 8 NeuronCores,
weights replicated.

Per-core device program (Tile/Bacc, fp32r matmuls):
 - upsample on DVE with a parity decomposition: for 2x align-corners
   resize 32->64, even outputs 2j = x[j] - (j/63)*(x[j]-x[j-1]) and odd
   outputs 2j+1 = x[j] + ((31-j)/63)*(x[j+1]-x[j]) -- affine-indexed
   stencils, computed in output-row quarters so temporaries stay small
   and conv0 can start after the first quarters land.
 - conv3x3 via 9 accumulating fp32r matmuls per output tile in a
   stride-65 "shared side pad" spatial layout: flat(y, x) = y*65 + x,
   where the right pad of row y coincides with the left pad of row y+1
   (both zero).  Output rows tiled 6 at a time (N = 390 <= 512 PSUM).
 - BN+ReLU fused into the PSUM drain: scalar-engine activation
   out = Relu(psum * scale_c + shift_c), scale/shift from g/b/m/v on
   device.
 - conv weights streamed through 8 shared SBUF slots (one quarter-pair
   of output channels resident), quarter-pair interleaved emission with
   sweep-aligned rowgroups so early-row matmuls across two co-quarters
   overlap the upsample; ci-major matmul order frees slots early for
   prefetch.
 - 70 dummy warmup matmuls on zeroed scratch keep the PE p-state/HAM
   warm while the upsample prefix runs.
"""
import sys

if '/opt/trn_rl_repo' not in sys.path:
    sys.path.insert(0, '/opt/trn_rl_repo')

import numpy as np
import concourse.bacc as bacc
import concourse.mybir as mybir
from concourse.tile import TileContext
from concourse.bass_utils import run_bass_kernel_spmd

F32 = mybir.dt.float32
F32R = mybir.dt.float32r
ALU = mybir.AluOpType
ACTF = mybir.ActivationFunctionType
EPS = 1e-5

S = 65                    # padded row stride (shared side pads)
PAD_LEN = 66 * 65 + 2     # 4292: 66 rows + tail (incl final garbage elem)
ROW_TILES = [(r, 6) for r in range(0, 60, 6)] + [(60, 4)]   # (r0, nrows)
ROW_GROUPS = [(0, 1), (2, 3, 4), (5, 6), (7, 8, 9, 10)]
N_CORES = 8


def _v2(ap2d, offset, rows, rowstep, cols):
    """[128, rows, cols] strided view of a [128, L] AP starting at offset."""
    sl = ap2d[:, offset: offset + rows * rowstep]
    return sl.rearrange("p (r c) -> p r c", c=rowstep)[:, :, 0:cols]


def build_patterns() -> np.ndarray:
    k = np.arange(1, 32)
    ko = np.arange(0, 31)
    wxe = -(k / 63.0)            # [0:31]   even x: x[k] + wxe[k-1]*d[k-1]
    wxo = (31 - ko) / 63.0       # [31:62]  odd  x: x[k] + wxo[k]*d[k]
    wye = -(k / 63.0)            # [62:93]  even y
    wyo = (31 - ko) / 63.0       # [93:124] odd  y
    return np.concatenate([wxe, wxo, wye, wyo]).astype(np.float32)


def build_nc(skip_upsample=False, skip_conv=False):
    nc = bacc.Bacc(None, target_bir_lowering=True)

    xt_d = nc.dram_tensor("xt", [512, 1024], F32, kind="ExternalInput")
    xm_d = nc.dram_tensor("xm", [512, 4096], F32, kind="ExternalInput")
    alpha_d = nc.dram_tensor("alpha", [1], F32, kind="ExternalInput")
    pat_d = nc.dram_tensor("pat", [124], F32, kind="ExternalInput")
    w0_d = nc.dram_tensor("w0r", [4 * 512, 9 * 128], F32R, kind="ExternalInput")
    w1_d = nc.dram_tensor("w1r", [2 * 512, 9 * 128], F32R, kind="ExternalInput")
    bn_d = {}
    for nm in ("g0", "b0", "m0", "v0"):
        bn_d[nm] = nc.dram_tensor(nm, [512, 1], F32, kind="ExternalInput")
    for nm in ("g1", "b1", "m1", "v1"):
        bn_d[nm] = nc.dram_tensor(nm, [256, 1], F32, kind="ExternalInput")
    out_d = nc.dram_tensor("out", [256, 4096], F32, kind="ExternalOutput")

    with TileContext(nc) as tc:
        with tc.tile_pool(name="main", bufs=1) as P, \
             tc.tile_pool(name="wp", bufs=8) as WP, \
             tc.tile_pool(name="xmp", bufs=2) as XMP, \
             tc.tile_pool(name="xtp", bufs=2) as XTP, \
             tc.tile_pool(name="outp", bufs=3) as OUTP, \
             tc.tile_pool(name="psum", bufs=8, space="PSUM") as PS:

            # ---------------- setup ----------------
            # first conv0 weight quarter: DMA'd before anything else so the
            # PE can start the moment the first xpad rows land
            w_prefetch = []
            first_feed = {}
            for ci in range(4):
                wt0 = WP.tile([128, 9 * 128], F32R, tag="w", name=f"wpre{ci}")
                nc.sync.dma_start(wt0[:], w0_d[ci * 128:(ci + 1) * 128, :])
                w_prefetch.append(wt0)
                if ci == 0 and not skip_upsample:
                    # first upsample chain's inputs right behind slot ci0 so
                    # the DVE starts ~6us earlier; remaining slots follow
                    xt00 = XTP.tile([128, 352], F32, tag="xtq", name="xt00")
                    nc.sync.dma_start(xt00[:, 0:9 * 32], xt_d[0:128, 0:9 * 32])
                    xm00 = XMP.tile([128, 1088], F32, tag="xm", name="xm00")
                    nc.sync.dma_start(xm00[:, 0:1024], xm_d[0:128, 0:1024])
                    first_feed = {"xt": xt00, "xm": xm00}

            # PE warmup: dummy fp32r matmuls on zeroed scratch keep the
            # PE p-state/HAM warm while the upsample prefix runs
            wscr = P.tile([128, 640], F32R, tag="wscr")
            nc.gpsimd.memset(wscr[:].bitcast(F32), 0.0)
            wsink = nc.dram_tensor("warm_sink", [128, 512], F32, kind="Internal")
            pw = PS.tile([128, 512], F32, tag="cpsum", name="pwarm")
            for wi in range(40):
                nc.tensor.matmul(pw[:], wscr[:, 0:128], wscr[:, 128:640],
                                 start=True, stop=True, skip_group_check=True)
            pat = P.tile([128, 124], F32, tag="pat")
            nc.sync.dma_start(pat[:], pat_d[:].partition_broadcast(128))
            alpha = P.tile([128, 1], F32, tag="alpha")
            nc.sync.dma_start(alpha[:], alpha_d[:].partition_broadcast(128))

            # BN scale/shift per co-tile: scale = g/sqrt(v+eps),
            # shift = b - m*scale.  Emitted after the first upsample sweep
            # (see below) so the tiny DVE ops don't delay conv0's inputs.
            bn_scale = {}
            bn_shift = {}
            def emit_bn():
                for layer, n_cot in ((0, 4), (1, 2)):
                  for q in range(n_cot):
                      g = P.tile([128, 1], F32, tag="bng")
                      b = P.tile([128, 1], F32, tag="bnb")
                      m = P.tile([128, 1], F32, tag="bnm")
                      v = P.tile([128, 1], F32, tag="bnv")
                      sl = slice(q * 128, (q + 1) * 128)
                      nc.sync.dma_start(g[:], bn_d[f"g{layer}"][sl, :])
                      nc.sync.dma_start(b[:], bn_d[f"b{layer}"][sl, :])
                      nc.sync.dma_start(m[:], bn_d[f"m{layer}"][sl, :])
                      nc.sync.dma_start(v[:], bn_d[f"v{layer}"][sl, :])
                      sc = P.tile([128, 1], F32, tag=f"sc{layer}_{q}")
                      sh = P.tile([128, 1], F32, tag=f"sh{layer}_{q}")
                      t = P.tile([128, 1], F32, tag="bnt")
                      nc.vector.tensor_scalar_add(t[:], v[:], EPS)
                      nc.scalar.activation(t[:], t[:], ACTF.Sqrt)
                      nc.vector.reciprocal(t[:], t[:])
                      nc.vector.tensor_mul(sc[:], g[:], t[:])
                      nc.vector.tensor_mul(t[:], m[:], sc[:])
                      nc.vector.tensor_sub(sh[:], b[:], t[:])
                      bn_scale[(layer, q)] = sc
                      bn_shift[(layer, q)] = sh

            # padded activation planes (fp32r so matmuls accept them)
            xpad = [P.tile([128, PAD_LEN], F32R, tag=f"xpad{i}", name=f"xpad{i}") for i in range(4)]
            ypad = [P.tile([128, PAD_LEN], F32R, tag=f"ypad{i}", name=f"ypad{i}") for i in range(4)]
            for t_ in xpad + ypad:
                nc.gpsimd.memset(t_[:, 0:65].bitcast(F32), 0.0)         # top row
                col = t_[:, 0:4290].rearrange("p (r c) -> p r c", c=65)[:, :, 0:1]
                nc.gpsimd.memset(col.bitcast(F32), 0.0)                 # side pads
                nc.gpsimd.memset(t_[:, 4226:PAD_LEN].bitcast(F32), 0.0) # bottom+tail


            # ---------------- phase A: upsample + fuse ----------------
            # out-row quarters [2*j0, 2*j0+16); per quarter per ci-tile the
            # x-interp is recomputed on just input rows [j0-1, j0+9).
            for j0 in (() if skip_upsample else (0, 8, 16, 24)):
                j1 = j0 + 8
                jstart = max(j0 - 1, 0)
                jstop2 = min(j1 + 1, 32)       # xh rows [jstart, jstop2)
                ny = jstop2 - jstart
                for ct in range(4):
                    eng = nc.vector
                    if j0 == 0 and ct == 0:
                        xt_t = first_feed["xt"]
                    else:
                        xt_t = XTP.tile([128, 352], F32, tag="xtq")
                        nc.sync.dma_start(
                            xt_t[:, 0:ny * 32],
                            xt_d[ct * 128:(ct + 1) * 128,
                                 jstart * 32: jstart * 32 + ny * 32])
                    # alpha * xm for this quarter's 16 output rows
                    if j0 == 0 and ct == 0:
                        xm_sb = first_feed["xm"]
                    else:
                        xm_sb = XMP.tile([128, 1088], F32, tag="xm")  # +64 slack
                        nc.sync.dma_start(
                            xm_sb[:, 0:1024],
                            xm_d[ct * 128:(ct + 1) * 128,
                                 2 * j0 * 64: 2 * j0 * 64 + 1024])
                    # --- x-interp on rows [jstart, jstop2) ---
                    d = P.tile([128, 10 * 31], F32, tag="dtmp")
                    dv = d[:, 0:ny * 31].rearrange("p (y k) -> p y k", k=31)
                    eng.tensor_sub(dv,
                                         _v2(xt_t, 1, ny, 32, 31),
                                         _v2(xt_t, 0, ny, 32, 31))
                    xh = P.tile([128, 10 * 64], F32, tag="xh")
                    xh4 = xh[:, 0:ny * 64].rearrange("p (y k t) -> p y k t",
                                                     k=32, t=2)
                    tx = P.tile([128, 10 * 31], F32, tag="ttmp")
                    txv = tx[:, 0:ny * 31].rearrange("p (y k) -> p y k", k=31)
                    # even cols 2k (k=1..31): xh = x[k] + wxe[k-1]*d[k-1]
                    pxe = pat[:, 0:31].unsqueeze(1).broadcast_to((128, ny, 31))
                    eng.tensor_mul(txv, dv, pxe)
                    eng.tensor_add(xh4[:, :, 1:32, 0:1].squeeze(),
                                         _v2(xt_t, 1, ny, 32, 31),
                                         txv)
                    eng.tensor_copy(xh4[:, :, 0:1, 0:1].squeeze(),
                                          _v2(xt_t, 0, ny, 32, 1).squeeze())
                    # odd cols 2k+1 (k=0..30): xh = x[k] + wxo[k]*d[k]
                    pxo = pat[:, 31:62].unsqueeze(1).broadcast_to((128, ny, 31))
                    eng.tensor_mul(txv, dv, pxo)
                    eng.tensor_add(xh4[:, :, 0:31, 1:2].squeeze(),
                                         _v2(xt_t, 0, ny, 32, 31),
                                         txv)
                    eng.tensor_copy(xh4[:, :, 31:32, 1:2].squeeze(),
                                          _v2(xt_t, 31, ny, 32, 1).squeeze())
                    # --- y-interp: D[j'] = xh[j'+1] - xh[j'] ---
                    nD = ny - 1                 # D rows [jstart, jstop2-1)
                    D = P.tile([128, 9 * 64], F32, tag="Dtmp")
                    eng.tensor_sub(D[:, 0:nD * 64],
                                         xh[:, 64: 64 + nD * 64],
                                         xh[:, 0: nD * 64])
                    ty = P.tile([128, 8 * 64], F32, tag="tytmp")
                    # even rows Y=2j, j in [max(j0,1), j1):
                    #   out = xh[j] + wye[j-1]*D[j-1] + xms
                    jlo = max(j0, 1)
                    n = j1 - jlo
                    tyv = ty[:, 0:n * 64].rearrange("p (r c) -> p r c", c=64)
                    pye = pat[:, 62 + jlo - 1: 62 + jlo - 1 + n] \
                        .unsqueeze(2).broadcast_to((128, n, 64))
                    eng.tensor_mul(
                        tyv, _v2(D, (jlo - 1 - jstart) * 64, n, 64, 64), pye)
                    eng.scalar_tensor_tensor(
                        tyv, _v2(xm_sb, (2 * jlo - 2 * j0) * 64, n, 128, 64),
                        alpha[:, 0:1], tyv, ALU.mult, ALU.add)
                    eng.tensor_add(
                        _v2(xpad[ct], (2 * jlo + 1) * S + 1, n, 2 * S, 64),
                        _v2(xh, (jlo - jstart) * 64, n, 64, 64), tyv)
                    if j0 == 0:
                        eng.scalar_tensor_tensor(
                            xpad[ct][:, S + 1: S + 1 + 64], xm_sb[:, 0:64],
                            alpha[:, 0:1], xh[:, 0:64], ALU.mult, ALU.add)
                    # odd rows Y=2j+1, j in [j0, min(j1,31)):
                    #   out = xh[j] + wyo[j]*D[j] + xms
                    jhi = min(j1, 31)
                    n = jhi - j0
                    tyv = ty[:, 0:n * 64].rearrange("p (r c) -> p r c", c=64)
                    pyo = pat[:, 93 + j0: 93 + j0 + n] \
                        .unsqueeze(2).broadcast_to((128, n, 64))
                    eng.tensor_mul(
                        tyv, _v2(D, (j0 - jstart) * 64, n, 64, 64), pyo)
                    eng.scalar_tensor_tensor(
                        tyv, _v2(xm_sb, 64, n, 128, 64),
                        alpha[:, 0:1], tyv, ALU.mult, ALU.add)
                    eng.tensor_add(
                        _v2(xpad[ct], (2 * j0 + 2) * S + 1, n, 2 * S, 64),
                        _v2(xh, (j0 - jstart) * 64, n, 64, 64), tyv)
                    if j1 == 32:
                        eng.scalar_tensor_tensor(
                            xpad[ct][:, 64 * S + 1: 64 * S + 1 + 64],
                            xm_sb[:, 15 * 64: 15 * 64 + 64],
                            alpha[:, 0:1],
                            xh[:, (31 - jstart) * 64: (31 - jstart) * 64 + 64],
                            ALU.mult, ALU.add)
                if j0 == 0:
                    emit_bn()

            if skip_upsample:
                emit_bn()

            # ---------------- conv layers ----------------
            def conv_layer(layer, n_cot, src, w_dram, w_cols, drain):
                for qp in range(0, n_cot, 2):
                    qs = [q for q in (qp, qp + 1) if q < n_cot]
                    wsl = {}
                    for q in qs:
                        if layer == 0 and q == 0:
                            wsl[q] = w_prefetch
                            continue
                        wsl[q] = []
                        for ci in range(4):
                            wt = WP.tile([128, 9 * 128], F32R, tag="w",
                                         name=f"wt_{layer}_{q}_{ci}")
                            row0 = (q * 4 + ci) * 128
                            nc.sync.dma_start(wt[:], w_dram[row0: row0 + 128, :])
                            wsl[q].append(wt)
                    # rowgroup-major across the quarter pair: both quarters'
                    # low-row groups run before any high-row group, so early
                    # PE work only depends on the first upsample sweeps
                    if layer == 0 and qp == 0:
                        rgs = ROW_GROUPS
                    elif layer == 0:
                        rgs = [(0, 1, 2, 3), (4, 5, 6, 7), (8, 9, 10)]
                    else:
                        rgs = [(0, 1, 2, 3), (4, 5, 6, 7), (8, 9), (10,)]
                    for rg in rgs:
                        for q in qs:
                            ptiles = []
                            for rt in rg:
                                r0, nr = ROW_TILES[rt]
                                pt = PS.tile([128, nr * 64], F32, tag="cpsum",
                                             name=f"ps_{layer}_{q}_{rt}")
                                ptiles.append(pt)
                            for ci in range(4):
                                for tap in range(9):
                                    dy, dx = divmod(tap, 3)
                                    lhsT = wsl[q][ci][:, tap * 128:(tap + 1) * 128]
                                    for i, rt in enumerate(rg):
                                        r0, nr = ROW_TILES[rt]
                                        off = (r0 + dy) * S + dx
                                        nc.tensor.matmul(
                                            ptiles[i][:, 0:nr * 64]
                                            .rearrange("p (r c) -> p r c", c=64),
                                            lhsT,
                                            _v2(src[ci], off, nr, S, 64),
                                            start=(ci == 0 and tap == 0),
                                            stop=(ci == 3 and tap == 8),
                                            skip_group_check=True)
                            for i, rt in enumerate(rg):
                                drain(q, rt, ptiles[i])

            def drain0(q, rt, ptile):
                r0, nr = ROW_TILES[rt]
                nc.scalar.activation(
                    _v2(ypad[q], (r0 + 1) * S + 1, nr, S, 64),
                    ptile[:, 0:nr * 64].rearrange("p (r c) -> p r c", c=64),
                    ACTF.Relu,
                    bias=bn_shift[(0, q)][:, 0:1],
                    scale=bn_scale[(0, q)][:, 0:1])

            def drain1(q, rt, ptile):
                r0, nr = ROW_TILES[rt]
                h1 = nr // 2
                for c0, cn in ((0, h1), (h1, nr - h1)):
                    ob = OUTP.tile([128, 192], F32, tag="ob", name=f"ob{q}_{rt}_{c0}")
                    nc.scalar.activation(
                        ob[:, 0:cn * 64],
                        ptile[:, c0 * 64: (c0 + cn) * 64],
                        ACTF.Relu,
                        bias=bn_shift[(1, q)][:, 0:1],
                        scale=bn_scale[(1, q)][:, 0:1])
                    nc.sync.dma_start(
                        out_d[q * 128:(q + 1) * 128,
                              (r0 + c0) * 64:(r0 + c0 + cn) * 64],
                        ob[:, 0:cn * 64])

            if not skip_conv:
                conv_layer(0, 4, xpad, w0_d, 512, drain0)
                conv_layer(1, 2, ypad, w1_d, 256, drain1)

    nc.finalize()
    return nc


_CACHED_NC = None


def _get_nc():
    global _CACHED_NC
    if _CACHED_NC is None:
        _CACHED_NC = build_nc()
    return _CACHED_NC


def kernel(**inputs) -> np.ndarray:
    xt = np.ascontiguousarray(np.asarray(inputs["xt"], np.float32))   # [8,512,32,32]
    xm = np.ascontiguousarray(np.asarray(inputs["xm"], np.float32))   # [8,512,64,64]
    alpha = np.asarray(inputs["alpha"], np.float32).reshape(1)
    w0 = np.asarray(inputs["w0"], np.float32)                         # [512,512,3,3]
    w1 = np.asarray(inputs["w1"], np.float32)                         # [256,512,3,3]
    def pack(w, n_cot):   # [co, ci, 3, 3] -> [n_cot*4*128, 9*128] slot-contiguous
        ci = w.shape[1]
        a = w.transpose(1, 2, 3, 0).reshape(4, 128, 9, n_cot, 128)
        a = a.transpose(3, 0, 1, 2, 4)          # [q, ci_t, ci_in, tap, co_in]
        return np.ascontiguousarray(a).reshape(n_cot * 4 * 128, 9 * 128)

    w0r = pack(w0, 4)
    w1r = pack(w1, 2)
    pat = build_patterns()

    common = {"alpha": alpha, "pat": pat, "w0r": w0r, "w1r": w1r}
    for nm in ("g0", "b0", "m0", "v0"):
        common[nm] = np.asarray(inputs[nm], np.float32).reshape(512, 1)
    for nm in ("g1", "b1", "m1", "v1"):
        common[nm] = np.asarray(inputs[nm], np.float32).reshape(256, 1)

    in_maps = []
    for b in range(N_CORES):
        m = dict(common)
        m["xt"] = np.ascontiguousarray(xt[b].reshape(512, 1024))
        m["xm"] = np.ascontiguousarray(xm[b].reshape(512, 4096))
        in_maps.append(m)

    nc = _get_nc()
    res = run_bass_kernel_spmd(nc, in_maps, core_ids=list(range(N_CORES)))
    out = np.stack([res.results[b]["out"].reshape(256, 64, 64)
                    for b in range(N_CORES)], axis=0)
    return out.astype(np.float32)



# revision 4
# speedup vs baseline: 1.2369x; 1.2369x over previous
"""Trainium2 Bass kernel for nn_Decoder_51539607552479.

DecoderModule.forward: bilinear-upsample xt (32->64, align_corners) ->
xfuse = xup + alpha*xm -> conv3x3(512->512)+BN+ReLU -> conv3x3(512->256)
+BN+ReLU.  Pure data parallel: batch dim (8) across the 8 NeuronCores,
weights replicated.

Per-core device program (Tile/Bacc):
 - convs run on the PE in fp8e4m3 DoubleRow mode (2 k-tiles per matmul at
   0.5 cyc/col) with hi/lo split-compensation: x ~ (xh + xl)/16 and
   W ~ (Wh + Wl)/64 in fp8, accumulating Wh*xh + Wh*xl + Wl*xh (+ Wl*xl
   on the odd tap) in fp32 PSUM -- ~2.25x fewer PE cycles than fp32r at
   ~1.7e-3 rel error.  Per (ci, tap-set) 14 DR instructions: 4 hh tap
   pairs, (hh|hl) on tap 8, 8 (lh|hl) correction pairs, (lh|ll) on tap 8.
 - upsample on DVE with the parity decomposition (even 2j = x[j] -
   (j/63)*(x[j]-x[j-1]), odd 2j+1 = x[j] + ((31-j)/63)*(x[j+1]-x[j])),
   producing 16*xfuse per output-row quarter into a compact f32 tile; the
   x16 scale is folded into the y-pattern weights and a fused stt.
 - quantization chains: conv0 input: hi = fp8(t32) on Act, lo32 = t32 -
   hi on DVE, lo = fp8(lo32) on Pool.  conv0 drain: BN+ReLU+x16 folded
   into two Act activations (fp8 hi plane + f32 temp), then sub and lo
   convert on Pool.  conv1 drain: one Act activation to f32, DMA out.
 - activations live in stride-65 "shared side pad" planes (flat(y,x) =
   y*65 + x), hi and lo halves PAD_LEN apart so one DR access pattern
   [p, 2, rows, 64] reads both.
 - emission is interleaved quarter-by-quarter (Q0, rows 0-11 of all four
   co-tiles, Q1, rows 12-27, ...) so each engine's FIFO stays in
   dependency order; all 16 conv0 weight slots prefetched up front.
 - 40 dummy fp32r matmuls on zeroed scratch keep the PE p-state warm
   during the upsample prefix.
"""
import sys

if '/opt/trn_rl_repo' not in sys.path:
    sys.path.insert(0, '/opt/trn_rl_repo')

import dataclasses

import numpy as np
import ml_dtypes
import concourse.bacc as bacc
import concourse.mybir as mybir
from concourse.tile import TileContext
from concourse.bass_utils import run_bass_kernel_spmd

F32 = mybir.dt.float32
F32R = mybir.dt.float32r
F8 = mybir.dt.float8e4
ALU = mybir.AluOpType
ACTF = mybir.ActivationFunctionType
DR = mybir.MatmulPerfMode.DoubleRow
EPS = 1e-5

S = 65                    # padded row stride (shared side pads)
PAD_LEN = 66 * 65 + 2     # 4292: 66 rows + tail pad
SX = 16.0                 # activation quant scale (both conv inputs)
SW = 64.0                 # weight quant scale (both layers)
N_INSTR = 14              # DR instructions per (ci, tap set)
N_CORES = 8

ROW_TILES = [(r, 4) for r in range(0, 64, 4)]       # 16 x 4-row psum tiles
RG_QUARTER = [(0, 1, 2), (3, 4, 5, 6), (7, 8, 9, 10), (11, 12, 13), (14, 15)]
RG_FULL = [(0, 1, 2, 3), (4, 5, 6, 7), (8, 9, 10, 11), (12, 13, 14, 15)]

# DR instruction plan per ci: (kind, tap_a, tap_b)
#  hh:   W=(Wh[ta], Wh[tb]), X=(hi o_ta, hi o_tb)
#  hhl:  W=(Wh[8], Wh[8]),   X=(hi o8, lo o8)
#  corr: W=(Wl[t], Wh[t]),   X=(hi o_t, lo o_t)
#  ll:   W=(Wl[8], Wl[8]),   X=(hi o8, lo o8)
INSTR_PLAN = ([("hh", 0, 1), ("hh", 2, 3), ("hh", 4, 5), ("hh", 6, 7),
               ("hhl", 8, 8)]
              + [("corr", t, t) for t in range(8)]
              + [("ll", 8, 8)])[:N_INSTR]


def _v2(ap2d, offset, rows, rowstep, cols):
    """[128, rows, cols] strided view of a [128, L] AP starting at offset."""
    sl = ap2d[:, offset: offset + rows * rowstep]
    return sl.rearrange("p (r c) -> p r c", c=rowstep)[:, :, 0:cols]


def _dr_rhs(tile_ap, off, dstride, nr):
    """[p, 2, nr, 64] moving AP: dim1 = k-tile pair (stride dstride),
    rows stride S, 64 cols."""
    base = tile_ap[:, off:off + 1]
    part = list(base.ap[0])
    return dataclasses.replace(
        base, ap=[part, [dstride, 2], [S, nr], [1, 64]])


def _tap_off(t, r0):
    return (r0 + t // 3) * S + (t % 3)


def build_patterns() -> np.ndarray:
    k = np.arange(1, 32)
    ko = np.arange(0, 31)
    wxe = -(k / 63.0)                  # [0:31]   even x
    wxo = (31 - ko) / 63.0             # [31:62]  odd  x
    wye = -(k / 63.0) * SX             # [62:93]  even y (x16 folded in)
    wyo = (31 - ko) / 63.0 * SX        # [93:124] odd  y
    return np.concatenate([wxe, wxo, wye, wyo]).astype(np.float32)


def build_nc():
    nc = bacc.Bacc(None, target_bir_lowering=True)

    xt_d = nc.dram_tensor("xt", [512, 1024], F32, kind="ExternalInput")
    xm_d = nc.dram_tensor("xm", [512, 4096], F32, kind="ExternalInput")
    alpha_d = nc.dram_tensor("alpha", [1], F32, kind="ExternalInput")
    pat_d = nc.dram_tensor("pat", [124], F32, kind="ExternalInput")
    w0_d = nc.dram_tensor("w0p", [16 * 128, N_INSTR * 256], F8,
                          kind="ExternalInput")
    w1_d = nc.dram_tensor("w1p", [8 * 128, N_INSTR * 256], F8,
                          kind="ExternalInput")
    bn_d = {}
    for nm in ("g0", "b0", "m0", "v0"):
        bn_d[nm] = nc.dram_tensor(nm, [512, 1], F32, kind="ExternalInput")
    for nm in ("g1", "b1", "m1", "v1"):
        bn_d[nm] = nc.dram_tensor(nm, [256, 1], F32, kind="ExternalInput")
    out_d = nc.dram_tensor("out", [256, 4096], F32, kind="ExternalOutput")

    with TileContext(nc) as tc:
        with tc.tile_pool(name="main", bufs=1) as P, \
             tc.tile_pool(name="wp", bufs=16) as WP, \
             tc.tile_pool(name="xmp", bufs=2) as XMP, \
             tc.tile_pool(name="xtp", bufs=2) as XTP, \
             tc.tile_pool(name="t32p", bufs=2) as T32P, \
             tc.tile_pool(name="xl32p", bufs=2) as XL32P, \
             tc.tile_pool(name="y32p", bufs=4) as Y32P, \
             tc.tile_pool(name="yl32p", bufs=4) as YL32P, \
             tc.tile_pool(name="outp", bufs=3) as OUTP, \
             tc.tile_pool(name="psum", bufs=8, space="PSUM") as PS:

            # ---------------- setup ----------------
            # first upsample chain's inputs, then conv0 q0 weight slots
            xt00 = XTP.tile([128, 352], F32, tag="xtq", name="xt00")
            nc.sync.dma_start(xt00[:, 0:9 * 32], xt_d[0:128, 0:9 * 32])
            xm00 = XMP.tile([128, 1088], F32, tag="xm", name="xm00")
            nc.sync.dma_start(xm00[:, 0:1024], xm_d[0:128, 0:1024])
            first_feed = {"xt": xt00, "xm": xm00}

            wsl0 = {q: [] for q in range(4)}
            for ci in range(4):
                wt0 = WP.tile([128, N_INSTR * 256], F8, tag="w",
                              name=f"w0_0_{ci}")
                nc.sync.dma_start(wt0[:], w0_d[ci * 128:(ci + 1) * 128, :])
                wsl0[0].append(wt0)

            # PE warmup: dummy fp32r matmuls keep the p-state warm while the
            # upsample prefix runs
            wscr = P.tile([128, 640], F32R, tag="wscr")
            nc.gpsimd.memset(wscr[:].bitcast(F32), 0.0)
            pw = PS.tile([128, 512], F32, tag="cpsum", name="pwarm")
            for wi in range(40):
                nc.tensor.matmul(pw[:], wscr[:, 0:128], wscr[:, 128:640],
                                 start=True, stop=True, skip_group_check=True)

            pat = P.tile([128, 124], F32, tag="pat")
            nc.sync.dma_start(pat[:], pat_d[:].partition_broadcast(128))
            alpha = P.tile([128, 1], F32, tag="alpha")
            nc.sync.dma_start(alpha[:], alpha_d[:].partition_broadcast(128))
            alpha16 = P.tile([128, 1], F32, tag="alpha16")
            nc.vector.tensor_scalar_mul(alpha16[:], alpha[:], SX)

            # padded fp8 activation planes: hi at 0, lo at PAD_LEN
            xpad = [P.tile([128, 2 * PAD_LEN], F8, tag=f"xpad{i}",
                           name=f"xpad{i}") for i in range(4)]
            ypad = [P.tile([128, 2 * PAD_LEN], F8, tag=f"ypad{i}",
                           name=f"ypad{i}") for i in range(4)]
            for t_ in xpad + ypad:
                nc.gpsimd.memset(t_[:].bitcast(F32), 0.0)

            # BN scale/shift per co-tile, with the fp8 scales folded in:
            #   conv0 drain:  y*SX = psum*(sc0/SW) + sh0*SX
            #   conv1 drain:  out  = psum*(sc1/(SX*SW)) + sh1
            bn_scale = {}
            bn_shift = {}
            def emit_bn():
                for layer, n_cot in ((0, 4), (1, 2)):
                  for q in range(n_cot):
                      g = P.tile([128, 1], F32, tag="bng")
                      b = P.tile([128, 1], F32, tag="bnb")
                      m = P.tile([128, 1], F32, tag="bnm")
                      v = P.tile([128, 1], F32, tag="bnv")
                      sl = slice(q * 128, (q + 1) * 128)
                      nc.sync.dma_start(g[:], bn_d[f"g{layer}"][sl, :])
                      nc.sync.dma_start(b[:], bn_d[f"b{layer}"][sl, :])
                      nc.sync.dma_start(m[:], bn_d[f"m{layer}"][sl, :])
                      nc.sync.dma_start(v[:], bn_d[f"v{layer}"][sl, :])
                      sc = P.tile([128, 1], F32, tag=f"sc{layer}_{q}")
                      sh = P.tile([128, 1], F32, tag=f"sh{layer}_{q}")
                      t = P.tile([128, 1], F32, tag="bnt")
                      nc.vector.tensor_scalar_add(t[:], v[:], EPS)
                      nc.scalar.activation(t[:], t[:], ACTF.Sqrt)
                      nc.vector.reciprocal(t[:], t[:])
                      nc.vector.tensor_mul(sc[:], g[:], t[:])
                      nc.vector.tensor_mul(t[:], m[:], sc[:])
                      nc.vector.tensor_sub(sh[:], b[:], t[:])
                      if layer == 0:
                          nc.vector.tensor_scalar_mul(sc[:], sc[:], 1.0 / SW)
                          nc.vector.tensor_scalar_mul(sh[:], sh[:], SX)
                      else:
                          nc.vector.tensor_scalar_mul(sc[:], sc[:],
                                                      1.0 / (SX * SW))
                      bn_scale[(layer, q)] = sc
                      bn_shift[(layer, q)] = sh

            # ------------- phase A quarter: upsample + fuse + quantize -----
            # out-row quarter [2*j0, 2*j0+16); t32 = 16*xfuse rows, compact
            def emit_quarter(j0):
                j1 = j0 + 8
                jstart = max(j0 - 1, 0)
                jstop2 = min(j1 + 1, 32)       # xh rows [jstart, jstop2)
                ny = jstop2 - jstart
                for ct in range(4):
                    eng = nc.vector
                    if j0 == 0 and ct == 0:
                        xt_t = first_feed["xt"]
                    else:
                        xt_t = XTP.tile([128, 352], F32, tag="xtq")
                        nc.sync.dma_start(
                            xt_t[:, 0:ny * 32],
                            xt_d[ct * 128:(ct + 1) * 128,
                                 jstart * 32: jstart * 32 + ny * 32])
                    if j0 == 0 and ct == 0:
                        xm_sb = first_feed["xm"]
                    else:
                        xm_sb = XMP.tile([128, 1088], F32, tag="xm")
                        nc.sync.dma_start(
                            xm_sb[:, 0:1024],
                            xm_d[ct * 128:(ct + 1) * 128,
                                 2 * j0 * 64: 2 * j0 * 64 + 1024])
                    # --- x-interp on rows [jstart, jstop2) (unscaled) ---
                    d = P.tile([128, 10 * 31], F32, tag="dtmp")
                    dv = d[:, 0:ny * 31].rearrange("p (y k) -> p y k", k=31)
                    eng.tensor_sub(dv,
                                   _v2(xt_t, 1, ny, 32, 31),
                                   _v2(xt_t, 0, ny, 32, 31))
                    xh = P.tile([128, 10 * 64], F32, tag="xh")
                    xh4 = xh[:, 0:ny * 64].rearrange("p (y k t) -> p y k t",
                                                     k=32, t=2)
                    tx = P.tile([128, 10 * 31], F32, tag="ttmp")
                    txv = tx[:, 0:ny * 31].rearrange("p (y k) -> p y k", k=31)
                    pxe = pat[:, 0:31].unsqueeze(1).broadcast_to((128, ny, 31))
                    eng.tensor_mul(txv, dv, pxe)
                    eng.tensor_add(xh4[:, :, 1:32, 0:1].squeeze(),
                                   _v2(xt_t, 1, ny, 32, 31),
                                   txv)
                    eng.tensor_copy(xh4[:, :, 0:1, 0:1].squeeze(),
                                    _v2(xt_t, 0, ny, 32, 1).squeeze())
                    pxo = pat[:, 31:62].unsqueeze(1).broadcast_to((128, ny, 31))
                    eng.tensor_mul(txv, dv, pxo)
                    eng.tensor_add(xh4[:, :, 0:31, 1:2].squeeze(),
                                   _v2(xt_t, 0, ny, 32, 31),
                                   txv)
                    eng.tensor_copy(xh4[:, :, 31:32, 1:2].squeeze(),
                                    _v2(xt_t, 31, ny, 32, 1).squeeze())
                    # --- y-interp into t32 = 16*(xh_y + wy*D) + 16*a*xm ---
                    nD = ny - 1
                    D = P.tile([128, 9 * 64], F32, tag="Dtmp")
                    eng.tensor_sub(D[:, 0:nD * 64],
                                   xh[:, 64: 64 + nD * 64],
                                   xh[:, 0: nD * 64])
                    t32 = T32P.tile([128, 1088], F32, tag="t32")
                    ty = P.tile([128, 8 * 64], F32, tag="tytmp")
                    # even rows Y=2j, j in [max(j0,1), j1): local row 2j-2j0
                    jlo = max(j0, 1)
                    n = j1 - jlo
                    tyv = ty[:, 0:n * 64].rearrange("p (r c) -> p r c", c=64)
                    pye = pat[:, 62 + jlo - 1: 62 + jlo - 1 + n] \
                        .unsqueeze(2).broadcast_to((128, n, 64))
                    eng.tensor_mul(
                        tyv, _v2(D, (jlo - 1 - jstart) * 64, n, 64, 64), pye)
                    eng.scalar_tensor_tensor(
                        tyv, _v2(xm_sb, (2 * jlo - 2 * j0) * 64, n, 128, 64),
                        alpha16[:, 0:1], tyv, ALU.mult, ALU.add)
                    eng.scalar_tensor_tensor(
                        _v2(t32, (2 * jlo - 2 * j0) * 64, n, 128, 64),
                        _v2(xh, (jlo - jstart) * 64, n, 64, 64),
                        SX, tyv, ALU.mult, ALU.add)
                    if j0 == 0:
                        bt = P.tile([128, 64], F32, tag="btmp")
                        eng.tensor_scalar_mul(bt[:], xh[:, 0:64], SX)
                        eng.scalar_tensor_tensor(
                            t32[:, 0:64], xm_sb[:, 0:64],
                            alpha16[:, 0:1], bt[:], ALU.mult, ALU.add)
                    # odd rows Y=2j+1, j in [j0, min(j1,31)): local 2j+1-2j0
                    jhi = min(j1, 31)
                    n = jhi - j0
                    tyv = ty[:, 0:n * 64].rearrange("p (r c) -> p r c", c=64)
                    pyo = pat[:, 93 + j0: 93 + j0 + n] \
                        .unsqueeze(2).broadcast_to((128, n, 64))
                    eng.tensor_mul(
                        tyv, _v2(D, (j0 - jstart) * 64, n, 64, 64), pyo)
                    eng.scalar_tensor_tensor(
                        tyv, _v2(xm_sb, 64, n, 128, 64),
                        alpha16[:, 0:1], tyv, ALU.mult, ALU.add)
                    eng.scalar_tensor_tensor(
                        _v2(t32, 64, n, 128, 64),
                        _v2(xh, (j0 - jstart) * 64, n, 64, 64),
                        SX, tyv, ALU.mult, ALU.add)
                    if j1 == 32:
                        bt = P.tile([128, 64], F32, tag="btmp")
                        eng.tensor_scalar_mul(
                            bt[:],
                            xh[:, (31 - jstart) * 64: (31 - jstart) * 64 + 64],
                            SX)
                        eng.scalar_tensor_tensor(
                            t32[:, 15 * 64: 16 * 64],
                            xm_sb[:, 15 * 64: 15 * 64 + 64],
                            alpha16[:, 0:1], bt[:], ALU.mult, ALU.add)
                    # --- quantize to fp8 hi/lo padded planes ---
                    hi_view = _v2(xpad[ct], (2 * j0 + 1) * S + 1, 16, S, 64)
                    lo_view = _v2(xpad[ct],
                                  PAD_LEN + (2 * j0 + 1) * S + 1, 16, S, 64)
                    t32v = _v2(t32, 0, 16, 64, 64)
                    nc.scalar.copy(hi_view, t32v)
                    xl32 = XL32P.tile([128, 1024], F32, tag="xl32")
                    xl32v = _v2(xl32, 0, 16, 64, 64)
                    nc.vector.tensor_sub(xl32v, t32v, hi_view)
                    nc.gpsimd.tensor_copy(lo_view, xl32v)

            # ---------------- conv pieces ----------------
            def emit_rg(rg, qs, src, wsl, drain):
                for q in qs:
                    ptiles = []
                    for rt in rg:
                        r0, nr = ROW_TILES[rt]
                        pt = PS.tile([128, nr * 64], F32, tag="cpsum",
                                     name=f"ps_{q}_{rt}")
                        ptiles.append(pt)
                    for ci in range(4):
                        for i, (kind, ta, tb) in enumerate(INSTR_PLAN):
                            lhsT = wsl[q][ci][:, i * 256:(i + 1) * 256] \
                                .rearrange("p (two m) -> p two m", two=2)
                            for k, rt in enumerate(rg):
                                r0, nr = ROW_TILES[rt]
                                off = _tap_off(ta, r0)
                                if kind == "hh":
                                    dstride = _tap_off(tb, r0) - off
                                else:
                                    dstride = PAD_LEN
                                nc.tensor.matmul(
                                    ptiles[k][:, 0:nr * 64]
                                    .rearrange("p (r c) -> p r c", c=64),
                                    lhsT,
                                    _dr_rhs(src[ci], off, dstride, nr),
                                    start=(ci == 0 and i == 0),
                                    stop=(ci == 3 and i == N_INSTR - 1),
                                    perf_mode=DR,
                                    skip_group_check=True)
                    for k, rt in enumerate(rg):
                        drain(q, rt, ptiles[k])

            def drain0(q, rt, ptile):
                r0, nr = ROW_TILES[rt]
                pv = ptile[:, 0:nr * 64].rearrange("p (r c) -> p r c", c=64)
                hi_view = _v2(ypad[q], (r0 + 1) * S + 1, nr, S, 64)
                lo_view = _v2(ypad[q], PAD_LEN + (r0 + 1) * S + 1, nr, S, 64)
                nc.scalar.activation(hi_view, pv, ACTF.Relu,
                                     bias=bn_shift[(0, q)][:, 0:1],
                                     scale=bn_scale[(0, q)][:, 0:1])
                y32 = Y32P.tile([128, 256], F32, tag="y32")
                y32v = _v2(y32, 0, nr, 64, 64)
                nc.scalar.activation(y32v, pv, ACTF.Relu,
                                     bias=bn_shift[(0, q)][:, 0:1],
                                     scale=bn_scale[(0, q)][:, 0:1])
                yl32 = YL32P.tile([128, 256], F32, tag="yl32")
                yl32v = _v2(yl32, 0, nr, 64, 64)
                nc.gpsimd.tensor_sub(yl32v, y32v, hi_view)
                nc.gpsimd.tensor_copy(lo_view, yl32v)

            def drain1(q, rt, ptile):
                r0, nr = ROW_TILES[rt]
                ob = OUTP.tile([128, 256], F32, tag="ob",
                               name=f"ob{q}_{rt}")
                nc.scalar.activation(ob[:, 0:nr * 64],
                                     ptile[:, 0:nr * 64],
                                     ACTF.Relu,
                                     bias=bn_shift[(1, q)][:, 0:1],
                                     scale=bn_scale[(1, q)][:, 0:1])
                nc.sync.dma_start(
                    out_d[q * 128:(q + 1) * 128, r0 * 64:(r0 + nr) * 64],
                    ob[:, 0:nr * 64])

            # ---------------- interleaved schedule ----------------
            emit_quarter(0)
            emit_bn()
            # remaining conv0 weight slots
            for q in range(1, 4):
                for ci in range(4):
                    wt = WP.tile([128, N_INSTR * 256], F8, tag="w",
                                 name=f"w0_{q}_{ci}")
                    row0 = (q * 4 + ci) * 128
                    nc.sync.dma_start(wt[:], w0_d[row0: row0 + 128, :])
                    wsl0[q].append(wt)

            qs0 = (0, 1, 2, 3)
            emit_rg(RG_QUARTER[0], qs0, xpad, wsl0, drain0)
            emit_quarter(8)
            emit_rg(RG_QUARTER[1], qs0, xpad, wsl0, drain0)
            emit_quarter(16)
            emit_rg(RG_QUARTER[2], qs0, xpad, wsl0, drain0)
            emit_quarter(24)
            emit_rg(RG_QUARTER[3], qs0, xpad, wsl0, drain0)
            emit_rg(RG_QUARTER[4], qs0, xpad, wsl0, drain0)

            # conv1
            wsl1 = {q: [] for q in range(2)}
            for q in range(2):
                for ci in range(4):
                    wt = WP.tile([128, N_INSTR * 256], F8, tag="w",
                                 name=f"w1_{q}_{ci}")
                    row0 = (q * 4 + ci) * 128
                    nc.sync.dma_start(wt[:], w1_d[row0: row0 + 128, :])
                    wsl1[q].append(wt)
            for rg in RG_FULL:
                emit_rg(rg, (0, 1), ypad, wsl1, drain1)

    nc.finalize()
    return nc


_CACHED_NC = None


def _get_nc():
    global _CACHED_NC
    if _CACHED_NC is None:
        _CACHED_NC = build_nc()
    return _CACHED_NC


def _q8(x):
    return x.astype(ml_dtypes.float8_e4m3)


def pack_weights(w, n_cot):
    """[co, ci, 3, 3] f32 -> [n_cot*4*128, N_INSTR*256] fp8 hi/lo packed."""
    wh = _q8(w * SW)
    wl = _q8(w * SW - wh.astype(np.float32))
    out = np.zeros((n_cot, 4, 128, N_INSTR, 2, 128), ml_dtypes.float8_e4m3)
    whr = wh.reshape(n_cot, 128, 4, 128, 9)   # [q, co_in, ci_t, ci_in, tap]
    wlr = wl.reshape(n_cot, 128, 4, 128, 9)
    whT = whr.transpose(0, 2, 3, 4, 1)        # [q, ci_t, ci_in, tap, co_in]
    wlT = wlr.transpose(0, 2, 3, 4, 1)
    for i, (kind, ta, tb) in enumerate(INSTR_PLAN):
        if kind == "hh":
            out[:, :, :, i, 0] = whT[:, :, :, ta]
            out[:, :, :, i, 1] = whT[:, :, :, tb]
        elif kind == "hhl":
            out[:, :, :, i, 0] = whT[:, :, :, 8]
            out[:, :, :, i, 1] = whT[:, :, :, 8]
        elif kind == "corr":
            out[:, :, :, i, 0] = wlT[:, :, :, ta]
            out[:, :, :, i, 1] = whT[:, :, :, ta]
        else:  # ll
            out[:, :, :, i, 0] = wlT[:, :, :, 8]
            out[:, :, :, i, 1] = wlT[:, :, :, 8]
    return np.ascontiguousarray(
        out.reshape(n_cot * 4 * 128, N_INSTR * 256))


def kernel(**inputs) -> np.ndarray:
    xt = np.ascontiguousarray(np.asarray(inputs["xt"], np.float32))
    xm = np.ascontiguousarray(np.asarray(inputs["xm"], np.float32))
    alpha = np.asarray(inputs["alpha"], np.float32).reshape(1)
    w0 = np.asarray(inputs["w0"], np.float32)
    w1 = np.asarray(inputs["w1"], np.float32)

    w0p = pack_weights(w0, 4)
    w1p = pack_weights(w1, 2)
    pat = build_patterns()

    common = {"alpha": alpha, "pat": pat, "w0p": w0p, "w1p": w1p}
    for nm in ("g0", "b0", "m0", "v0"):
        common[nm] = np.asarray(inputs[nm], np.float32).reshape(512, 1)
    for nm in ("g1", "b1", "m1", "v1"):
        common[nm] = np.asarray(inputs[nm], np.float32).reshape(256, 1)

    in_maps = []
    for b in range(N_CORES):
        m = dict(common)
        m["xt"] = np.ascontiguousarray(xt[b].reshape(512, 1024))
        m["xm"] = np.ascontiguousarray(xm[b].reshape(512, 4096))
        in_maps.append(m)

    nc = _get_nc()
    res = run_bass_kernel_spmd(nc, in_maps, core_ids=list(range(N_CORES)))
    out = np.stack([res.results[b]["out"].reshape(256, 64, 64)
                    for b in range(N_CORES)], axis=0)
    return out.astype(np.float32)


# revision 14
# speedup vs baseline: 1.3254x; 1.0716x over previous
"""Trainium2 Bass kernel for nn_Decoder_51539607552479.

DecoderModule.forward: bilinear-upsample xt (32->64, align_corners) ->
xfuse = xup + alpha*xm -> conv3x3(512->512)+BN+ReLU -> conv3x3(512->256)
+BN+ReLU.  Pure data parallel: batch dim (8) across the 8 NeuronCores,
weights replicated.

Per-core device program (Tile/Bacc):
 - convs run on the PE in fp8e4m3 DoubleRow mode (2 k-tiles per matmul at
   0.5 cyc/col) with hi/lo split-compensation: x ~ (xh + xl)/16 and
   W ~ (Wh + Wl)/64 in fp8, accumulating Wh*xh + Wh*xl + Wl*xh (+ Wl*xl
   on the odd tap) in fp32 PSUM -- ~2.25x fewer PE cycles than fp32r at
   ~1.7e-3 rel error.  Per (ci, tap-set) 14 DR instructions: 4 hh tap
   pairs, (hh|hl) on tap 8, 8 (lh|hl) correction pairs, (lh|ll) on tap 8.
 - upsample on DVE with the parity decomposition (even 2j = x[j] -
   (j/63)*(x[j]-x[j-1]), odd 2j+1 = x[j] + ((31-j)/63)*(x[j+1]-x[j])),
   producing 16*xfuse per output-row quarter into a compact f32 tile; the
   x16 scale is folded into the y-pattern weights and a fused stt.
 - quantization chains: conv0 input: hi = fp8(t32) on Act, lo32 = t32 -
   hi on DVE, lo = fp8(lo32) on Pool.  conv0 drain: BN+ReLU+x16 folded
   into two Act activations (fp8 hi plane + f32 temp), then sub and lo
   convert on Pool.  conv1 drain: one Act activation to f32, DMA out.
 - activations live in stride-65 "shared side pad" planes (flat(y,x) =
   y*65 + x), hi and lo halves PAD_LEN apart so one DR access pattern
   [p, 2, rows, 64] reads both.
 - emission is interleaved quarter-by-quarter (Q0, rows 0-11 of all four
   co-tiles, Q1, rows 12-27, ...) so each engine's FIFO stays in
   dependency order; all 16 conv0 weight slots prefetched up front.
 - 40 dummy fp32r matmuls on zeroed scratch keep the PE p-state warm
   during the upsample prefix.
"""
import sys

if '/opt/trn_rl_repo' not in sys.path:
    sys.path.insert(0, '/opt/trn_rl_repo')

import dataclasses

import numpy as np
import ml_dtypes
import concourse.bacc as bacc
import concourse.mybir as mybir
from concourse.tile import TileContext
from concourse.bass_utils import run_bass_kernel_spmd

F32 = mybir.dt.float32
F32R = mybir.dt.float32r
F8 = mybir.dt.float8e4
ALU = mybir.AluOpType
ACTF = mybir.ActivationFunctionType
DR = mybir.MatmulPerfMode.DoubleRow
EPS = 1e-5

S = 65                    # padded row stride (shared side pads)
PAD_LEN = 66 * 65 + 2     # 4292: 66 rows + tail pad
SX = 16.0                 # activation quant scale (both conv inputs)
SW = 64.0                 # weight quant scale (both layers)
N_INSTR = 13              # DR instructions per (ci, tap set)
N_WARMUP = 68             # PE p-state warmup matmuls
N_CORES = 8

ROW_TILES = [(r, 4) for r in range(0, 64, 4)]       # 16 x 4-row psum tiles
QUARTERS = [(0, 9), (9, 17), (17, 25), (25, 32)]    # j-ranges, out rows 2ja..2jb
RG_FULL = [(0, 1, 2, 3), (4, 5, 6, 7), (8, 9, 10, 11), (12, 13, 14, 15)]

# DR instruction plan per ci: (kind, tap_a, tap_b)
#  hh:   W=(Wh[ta], Wh[tb]), X=(hi o_ta, hi o_tb)
#  hhl:  W=(Wh[8], Wh[8]),   X=(hi o8, lo o8)
#  corr: W=(Wl[t], Wh[t]),   X=(hi o_t, lo o_t)
#  ll:   W=(Wl[8], Wl[8]),   X=(hi o8, lo o8)
INSTR_PLAN = ([("hh", 0, 1), ("hh", 2, 3), ("hh", 4, 5), ("hh", 6, 7),
               ("hhl", 8, 8)]
              + [("corr", t, t) for t in range(8)]
              + [("ll", 8, 8)])[:N_INSTR]


def _v2(ap2d, offset, rows, rowstep, cols):
    """[128, rows, cols] strided view of a [128, L] AP starting at offset."""
    sl = ap2d[:, offset: offset + rows * rowstep]
    return sl.rearrange("p (r c) -> p r c", c=rowstep)[:, :, 0:cols]


def _dr_rhs(tile_ap, off, dstride, nr):
    """[p, 2, nr, 64] moving AP: dim1 = k-tile pair (stride dstride),
    rows stride S, 64 cols."""
    base = tile_ap[:, off:off + 1]
    part = list(base.ap[0])
    return dataclasses.replace(
        base, ap=[part, [dstride, 2], [S, nr], [1, 64]])


def _tap_off(t, r0):
    return (r0 + t // 3) * S + (t % 3)


def _pad_regions(plane):
    """APs covering the zero-pad bytes of both half-planes (hi+lo merged
    via a [PAD_LEN, 2] dim): top row, left-pad column, bottom row+tail."""
    import dataclasses as _dc
    def mk(off, dims):
        base = plane[:, off:off + 1]
        return _dc.replace(base, ap=[list(base.ap[0]), [PAD_LEN, 2]] + dims)
    return [mk(0, [[1, 65]]),            # row 0
            mk(65, [[S, 64], [1, 1]]),   # left pads rows 1..64
            mk(65 * S, [[1, 67]])]       # bottom row + tail


def build_patterns() -> np.ndarray:
    k = np.arange(1, 32)
    ko = np.arange(0, 31)
    wxe = -(k / 63.0)                  # [0:31]   even x
    wxo = (31 - ko) / 63.0             # [31:62]  odd  x
    wye = -(k / 63.0) * SX             # [62:93]  even y (x16 folded in)
    wyo = (31 - ko) / 63.0 * SX        # [93:124] odd  y
    return np.concatenate([wxe, wxo, wye, wyo]).astype(np.float32)


def build_nc():
    nc = bacc.Bacc(None, target_bir_lowering=True)

    xt_d = nc.dram_tensor("xt", [512, 1024], F32, kind="ExternalInput")
    xm_d = nc.dram_tensor("xm", [512, 4096], F32, kind="ExternalInput")
    alpha_d = nc.dram_tensor("alpha", [1], F32, kind="ExternalInput")
    pat_d = nc.dram_tensor("pat", [124], F32, kind="ExternalInput")
    w0_d = nc.dram_tensor("w0p", [16 * 128, N_INSTR * 256], F8,
                          kind="ExternalInput")
    w1_d = nc.dram_tensor("w1p", [8 * 128, N_INSTR * 256], F8,
                          kind="ExternalInput")
    bn_d = {}
    for nm in ("g0", "b0", "m0", "v0"):
        bn_d[nm] = nc.dram_tensor(nm, [512, 1], F32, kind="ExternalInput")
    for nm in ("g1", "b1", "m1", "v1"):
        bn_d[nm] = nc.dram_tensor(nm, [256, 1], F32, kind="ExternalInput")
    out_d = nc.dram_tensor("out", [256, 4096], F32, kind="ExternalOutput")

    with TileContext(nc) as tc:
        with tc.tile_pool(name="main", bufs=1) as P, \
             tc.tile_pool(name="wp", bufs=16) as WP, \
             tc.tile_pool(name="xmp", bufs=2) as XMP, \
             tc.tile_pool(name="xtp", bufs=2) as XTP, \
             tc.tile_pool(name="t32p", bufs=2) as T32P, \
             tc.tile_pool(name="xl32p", bufs=2) as XL32P, \
             tc.tile_pool(name="y32p", bufs=4) as Y32P, \
             tc.tile_pool(name="yl32p", bufs=4) as YL32P, \
             tc.tile_pool(name="outp", bufs=6) as OUTP, \
             tc.tile_pool(name="psum", bufs=8, space="PSUM") as PS:

            # ---------------- setup ----------------
            # first upsample chain's inputs, then conv0 q0 weight slots
            xt00 = XTP.tile([128, 352], F32, tag="xtq", name="xt00")
            nc.sync.dma_start(xt00[:, 0:10 * 32], xt_d[0:128, 0:10 * 32])
            xm00 = XMP.tile([128, 1216], F32, tag="xm", name="xm00")
            nc.sync.dma_start(xm00[:, 0:1152], xm_d[0:128, 0:1152])
            first_feed = {"xt": xt00, "xm": xm00}

            # preload Act function tables (Copy + Relu + Sqrt) off the
            # critical path so the first real activation doesn't pay the
            # LoadActFuncSet latency
            actw = P.tile([128, 1], F32, tag="actw")
            nc.vector.memset(actw[:], 0.0)
            nc.scalar.copy(actw[:], actw[:])
            nc.scalar.activation(actw[:], actw[:], ACTF.Relu)
            nc.scalar.activation(actw[:], actw[:], ACTF.Sqrt)

            pat = P.tile([128, 124], F32, tag="pat")
            nc.sync.dma_start(pat[:], pat_d[:].partition_broadcast(128))
            alpha = P.tile([128, 1], F32, tag="alpha")
            nc.sync.dma_start(alpha[:], alpha_d[:].partition_broadcast(128))
            alpha16 = P.tile([128, 1], F32, tag="alpha16")
            nc.vector.tensor_scalar_mul(alpha16[:], alpha[:], SX)

            wsl0 = {q: [] for q in range(4)}
            for ci in range(4):
                wt0 = WP.tile([128, N_INSTR * 256], F8, tag="w",
                              name=f"w0_0_{ci}")
                nc.sync.dma_start(wt0[:], w0_d[ci * 128:(ci + 1) * 128, :])
                wsl0[0].append(wt0)

            # PE warmup: dummy fp32r matmuls keep the p-state warm while the
            # upsample prefix runs
            wscr = P.tile([128, 640], F32R, tag="wscr")
            nc.gpsimd.memset(wscr[:].bitcast(F32), 0.0)
            pw = PS.tile([128, 512], F32, tag="cpsum", name="pwarm")
            for wi in range(N_WARMUP):
                nc.tensor.matmul(pw[:], wscr[:, 0:128], wscr[:, 128:640],
                                 start=(wi == 0), stop=(wi == N_WARMUP - 1),
                                 skip_group_check=True)

            # padded fp8 activation planes: hi at 0, lo at PAD_LEN.
            # Only the pad bytes are memset (interior is fully overwritten),
            # keeping Pool free for the quarter-0 quant chain.
            xpad = [P.tile([128, 2 * PAD_LEN], F8, tag=f"xpad{i}",
                           name=f"xpad{i}") for i in range(4)]
            ypad = [P.tile([128, 2 * PAD_LEN], F8, tag=f"ypad{i}",
                           name=f"ypad{i}") for i in range(4)]
            for t_ in xpad + ypad:
                for reg in _pad_regions(t_):
                    nc.gpsimd.memset(reg, 0.0)

            # BN scale/shift per co-tile, with the fp8 scales folded in:
            #   conv0 drain:  y*SX = psum*(sc0/SW) + sh0*SX
            #   conv1 drain:  out  = psum*(sc1/(SX*SW)) + sh1
            bn_scale = {}
            bn_shift = {}
            def emit_bn():
                for layer, n_cot in ((0, 4), (1, 2)):
                  for q in range(n_cot):
                      g = P.tile([128, 1], F32, tag="bng")
                      b = P.tile([128, 1], F32, tag="bnb")
                      m = P.tile([128, 1], F32, tag="bnm")
                      v = P.tile([128, 1], F32, tag="bnv")
                      sl = slice(q * 128, (q + 1) * 128)
                      nc.sync.dma_start(g[:], bn_d[f"g{layer}"][sl, :])
                      nc.sync.dma_start(b[:], bn_d[f"b{layer}"][sl, :])
                      nc.sync.dma_start(m[:], bn_d[f"m{layer}"][sl, :])
                      nc.sync.dma_start(v[:], bn_d[f"v{layer}"][sl, :])
                      sc = P.tile([128, 1], F32, tag=f"sc{layer}_{q}")
                      sh = P.tile([128, 1], F32, tag=f"sh{layer}_{q}")
                      t = P.tile([128, 1], F32, tag="bnt")
                      nc.vector.tensor_scalar_add(t[:], v[:], EPS)
                      nc.scalar.activation(t[:], t[:], ACTF.Sqrt)
                      nc.vector.reciprocal(t[:], t[:])
                      nc.vector.tensor_mul(sc[:], g[:], t[:])
                      nc.vector.tensor_mul(t[:], m[:], sc[:])
                      nc.vector.tensor_sub(sh[:], b[:], t[:])
                      if layer == 0:
                          nc.vector.tensor_scalar_mul(sc[:], sc[:], 1.0 / SW)
                          nc.vector.tensor_scalar_mul(sh[:], sh[:], SX)
                      else:
                          nc.vector.tensor_scalar_mul(sc[:], sc[:],
                                                      1.0 / (SX * SW))
                      bn_scale[(layer, q)] = sc
                      bn_shift[(layer, q)] = sh

            # ------------- phase A quarter: upsample + fuse + quantize -----
            # out-row quarter [2*j0, 2*j0+16); t32 = 16*xfuse rows, compact
            def emit_quarter(ja, jb):
                j0, j1 = ja, jb
                n_rows = 2 * (jb - ja)
                jstart = max(j0 - 1, 0)
                jstop2 = min(j1 + 1, 32)       # xh rows [jstart, jstop2)
                ny = jstop2 - jstart
                for ct in range(4):
                    eng = nc.vector
                    if j0 == 0 and ct == 0:
                        xt_t = first_feed["xt"]
                    else:
                        xt_t = XTP.tile([128, 352], F32, tag="xtq")
                        nc.sync.dma_start(
                            xt_t[:, 0:ny * 32],
                            xt_d[ct * 128:(ct + 1) * 128,
                                 jstart * 32: jstart * 32 + ny * 32])
                    if j0 == 0 and ct == 0:
                        xm_sb = first_feed["xm"]
                    else:
                        xm_sb = XMP.tile([128, 1216], F32, tag="xm")
                        nc.sync.dma_start(
                            xm_sb[:, 0:n_rows * 64],
                            xm_d[ct * 128:(ct + 1) * 128,
                                 2 * j0 * 64: (2 * j0 + n_rows) * 64])
                    # --- x-interp on rows [jstart, jstop2) (unscaled) ---
                    d = P.tile([128, 10 * 31], F32, tag="dtmp")
                    dv = d[:, 0:ny * 31].rearrange("p (y k) -> p y k", k=31)
                    eng.tensor_sub(dv,
                                   _v2(xt_t, 1, ny, 32, 31),
                                   _v2(xt_t, 0, ny, 32, 31))
                    xh = P.tile([128, 10 * 64], F32, tag="xh")
                    xh4 = xh[:, 0:ny * 64].rearrange("p (y k t) -> p y k t",
                                                     k=32, t=2)
                    tx = P.tile([128, 10 * 31], F32, tag="ttmp")
                    txv = tx[:, 0:ny * 31].rearrange("p (y k) -> p y k", k=31)
                    pxe = pat[:, 0:31].unsqueeze(1).broadcast_to((128, ny, 31))
                    eng.tensor_mul(txv, dv, pxe)
                    eng.tensor_add(xh4[:, :, 1:32, 0:1].squeeze(),
                                   _v2(xt_t, 1, ny, 32, 31),
                                   txv)
                    eng.tensor_copy(xh4[:, :, 0:1, 0:1].squeeze(),
                                    _v2(xt_t, 0, ny, 32, 1).squeeze())
                    pxo = pat[:, 31:62].unsqueeze(1).broadcast_to((128, ny, 31))
                    eng.tensor_mul(txv, dv, pxo)
                    eng.tensor_add(xh4[:, :, 0:31, 1:2].squeeze(),
                                   _v2(xt_t, 0, ny, 32, 31),
                                   txv)
                    eng.tensor_copy(xh4[:, :, 31:32, 1:2].squeeze(),
                                    _v2(xt_t, 31, ny, 32, 1).squeeze())
                    # --- y-interp into t32 = 16*(xh_y + wy*D) + 16*a*xm ---
                    nD = ny - 1
                    D = P.tile([128, 9 * 64], F32, tag="Dtmp")
                    eng.tensor_sub(D[:, 0:nD * 64],
                                   xh[:, 64: 64 + nD * 64],
                                   xh[:, 0: nD * 64])
                    t32 = T32P.tile([128, 1216], F32, tag="t32")
                    ty = P.tile([128, 9 * 64], F32, tag="tytmp")
                    # even rows Y=2j, j in [max(j0,1), j1): local row 2j-2j0
                    jlo = max(j0, 1)
                    n = j1 - jlo
                    tyv = ty[:, 0:n * 64].rearrange("p (r c) -> p r c", c=64)
                    pye = pat[:, 62 + jlo - 1: 62 + jlo - 1 + n] \
                        .unsqueeze(2).broadcast_to((128, n, 64))
                    eng.tensor_mul(
                        tyv, _v2(D, (jlo - 1 - jstart) * 64, n, 64, 64), pye)
                    eng.scalar_tensor_tensor(
                        tyv, _v2(xm_sb, (2 * jlo - 2 * j0) * 64, n, 128, 64),
                        alpha16[:, 0:1], tyv, ALU.mult, ALU.add)
                    eng.scalar_tensor_tensor(
                        _v2(t32, (2 * jlo - 2 * j0) * 64, n, 128, 64),
                        _v2(xh, (jlo - jstart) * 64, n, 64, 64),
                        SX, tyv, ALU.mult, ALU.add)
                    if j0 == 0:
                        bt = P.tile([128, 64], F32, tag="btmp")
                        eng.tensor_scalar_mul(bt[:], xh[:, 0:64], SX)
                        eng.scalar_tensor_tensor(
                            t32[:, 0:64], xm_sb[:, 0:64],
                            alpha16[:, 0:1], bt[:], ALU.mult, ALU.add)
                    # odd rows Y=2j+1, j in [j0, min(j1,31)): local 2j+1-2j0
                    jhi = min(j1, 31)
                    n = jhi - j0
                    tyv = ty[:, 0:n * 64].rearrange("p (r c) -> p r c", c=64)
                    pyo = pat[:, 93 + j0: 93 + j0 + n] \
                        .unsqueeze(2).broadcast_to((128, n, 64))
                    eng.tensor_mul(
                        tyv, _v2(D, (j0 - jstart) * 64, n, 64, 64), pyo)
                    eng.scalar_tensor_tensor(
                        tyv, _v2(xm_sb, 64, n, 128, 64),
                        alpha16[:, 0:1], tyv, ALU.mult, ALU.add)
                    eng.scalar_tensor_tensor(
                        _v2(t32, 64, n, 128, 64),
                        _v2(xh, (j0 - jstart) * 64, n, 64, 64),
                        SX, tyv, ALU.mult, ALU.add)
                    if j1 == 32:
                        bt = P.tile([128, 64], F32, tag="btmp")
                        eng.tensor_scalar_mul(
                            bt[:],
                            xh[:, (31 - jstart) * 64: (31 - jstart) * 64 + 64],
                            SX)
                        eng.scalar_tensor_tensor(
                            t32[:, (n_rows - 1) * 64: n_rows * 64],
                            xm_sb[:, (n_rows - 1) * 64: (n_rows - 1) * 64 + 64],
                            alpha16[:, 0:1], bt[:], ALU.mult, ALU.add)
                    # --- quantize to fp8 hi/lo padded planes ---
                    hi_view = _v2(xpad[ct], (2 * j0 + 1) * S + 1, n_rows, S, 64)
                    lo_view = _v2(xpad[ct],
                                  PAD_LEN + (2 * j0 + 1) * S + 1, n_rows, S, 64)
                    t32v = _v2(t32, 0, n_rows, 64, 64)
                    nc.scalar.copy(hi_view, t32v)
                    xl32 = XL32P.tile([128, 1216], F32, tag="xl32")
                    xl32v = _v2(xl32, 0, n_rows, 64, 64)
                    nc.vector.tensor_sub(xl32v, t32v, hi_view)
                    nc.gpsimd.tensor_copy(lo_view, xl32v)

            # ---------------- conv pieces ----------------
            def emit_rg(rg, qs, src, wsl, drain):
                for q in qs:
                    ptiles = []
                    for rt in rg:
                        r0, nr = ROW_TILES[rt]
                        pt = PS.tile([128, nr * 64], F32, tag="cpsum",
                                     name=f"ps_{q}_{rt}")
                        ptiles.append(pt)
                    for ci in range(4):
                        for i, (kind, ta, tb) in enumerate(INSTR_PLAN):
                            lhsT = wsl[q][ci][:, i * 256:(i + 1) * 256] \
                                .rearrange("p (two m) -> p two m", two=2)
                            for k, rt in enumerate(rg):
                                r0, nr = ROW_TILES[rt]
                                off = _tap_off(ta, r0)
                                if kind == "hh":
                                    dstride = _tap_off(tb, r0) - off
                                else:
                                    dstride = PAD_LEN
                                nc.tensor.matmul(
                                    ptiles[k][:, 0:nr * 64]
                                    .rearrange("p (r c) -> p r c", c=64),
                                    lhsT,
                                    _dr_rhs(src[ci], off, dstride, nr),
                                    start=(ci == 0 and i == 0),
                                    stop=(ci == 3 and i == N_INSTR - 1),
                                    perf_mode=DR,
                                    skip_group_check=True)
                    for k, rt in enumerate(rg):
                        drain(q, rt, ptiles[k])

            def drain0(q, rt, ptile):
                r0, nr = ROW_TILES[rt]
                pv = ptile[:, 0:nr * 64].rearrange("p (r c) -> p r c", c=64)
                hi_view = _v2(ypad[q], (r0 + 1) * S + 1, nr, S, 64)
                lo_view = _v2(ypad[q], PAD_LEN + (r0 + 1) * S + 1, nr, S, 64)
                nc.scalar.activation(hi_view, pv, ACTF.Relu,
                                     bias=bn_shift[(0, q)][:, 0:1],
                                     scale=bn_scale[(0, q)][:, 0:1])
                y32 = Y32P.tile([128, 256], F32, tag="y32")
                y32v = _v2(y32, 0, nr, 64, 64)
                nc.scalar.activation(y32v, pv, ACTF.Relu,
                                     bias=bn_shift[(0, q)][:, 0:1],
                                     scale=bn_scale[(0, q)][:, 0:1])
                yl32 = YL32P.tile([128, 256], F32, tag="yl32")
                yl32v = _v2(yl32, 0, nr, 64, 64)
                nc.gpsimd.tensor_sub(yl32v, y32v, hi_view)
                nc.gpsimd.tensor_copy(lo_view, yl32v)

            def drain1(q, rt, ptile):
                r0, nr = ROW_TILES[rt]
                ob = OUTP.tile([128, 256], F32, tag="ob",
                               name=f"ob{q}_{rt}")
                nc.scalar.activation(ob[:, 0:nr * 64],
                                     ptile[:, 0:nr * 64],
                                     ACTF.Relu,
                                     bias=bn_shift[(1, q)][:, 0:1],
                                     scale=bn_scale[(1, q)][:, 0:1])
                dma_eng = nc.scalar if rt % 2 else nc.sync
                dma_eng.dma_start(
                    out_d[q * 128:(q + 1) * 128, r0 * 64:(r0 + nr) * 64],
                    ob[:, 0:nr * 64])

            # ---------------- interleaved schedule ----------------
            def emit_w0_slots(q):
                for ci in range(4):
                    wt = WP.tile([128, N_INSTR * 256], F8, tag="w",
                                 name=f"w0_{q}_{ci}")
                    row0 = (q * 4 + ci) * 128
                    nc.sync.dma_start(wt[:], w0_d[row0: row0 + 128, :])
                    wsl0[q].append(wt)

            emit_quarter(*QUARTERS[0])
            emit_bn()
            # first rowgroup: weight-slot DMAs interleaved per co-tile so the
            # q0 matmuls only wait on q0's four DMAs
            emit_rg(RG_FULL[0], (0,), xpad, wsl0, drain0)
            emit_w0_slots(1)
            emit_rg(RG_FULL[0], (1,), xpad, wsl0, drain0)
            emit_w0_slots(2)
            emit_rg(RG_FULL[0], (2,), xpad, wsl0, drain0)
            emit_w0_slots(3)
            emit_rg(RG_FULL[0], (3,), xpad, wsl0, drain0)
            qs0 = (0, 1, 2, 3)
            emit_quarter(*QUARTERS[1])
            emit_rg(RG_FULL[1], qs0, xpad, wsl0, drain0)
            emit_quarter(*QUARTERS[2])
            emit_rg(RG_FULL[2], qs0, xpad, wsl0, drain0)
            emit_quarter(*QUARTERS[3])
            emit_rg(RG_FULL[3], qs0, xpad, wsl0, drain0)

            # conv1
            wsl1 = {q: [] for q in range(2)}
            def emit_w1_slots(q):
                for ci in range(4):
                    wt = WP.tile([128, N_INSTR * 256], F8, tag="w",
                                 name=f"w1_{q}_{ci}")
                    row0 = (q * 4 + ci) * 128
                    nc.sync.dma_start(wt[:], w1_d[row0: row0 + 128, :])
                    wsl1[q].append(wt)
            emit_w1_slots(0)
            emit_rg(RG_FULL[0], (0,), ypad, wsl1, drain1)
            emit_w1_slots(1)
            emit_rg(RG_FULL[0], (1,), ypad, wsl1, drain1)
            for rg in RG_FULL[1:]:
                emit_rg(rg, (0, 1), ypad, wsl1, drain1)

    nc.finalize()
    return nc


_CACHED_NC = None


def _get_nc():
    global _CACHED_NC
    if _CACHED_NC is None:
        _CACHED_NC = build_nc()
    return _CACHED_NC


def _q8(x):
    return x.astype(ml_dtypes.float8_e4m3)


def pack_weights(w, n_cot):
    """[co, ci, 3, 3] f32 -> [n_cot*4*128, N_INSTR*256] fp8 hi/lo packed."""
    wh = _q8(w * SW)
    wl = _q8(w * SW - wh.astype(np.float32))
    out = np.zeros((n_cot, 4, 128, N_INSTR, 2, 128), ml_dtypes.float8_e4m3)
    whr = wh.reshape(n_cot, 128, 4, 128, 9)   # [q, co_in, ci_t, ci_in, tap]
    wlr = wl.reshape(n_cot, 128, 4, 128, 9)
    whT = whr.transpose(0, 2, 3, 4, 1)        # [q, ci_t, ci_in, tap, co_in]
    wlT = wlr.transpose(0, 2, 3, 4, 1)
    for i, (kind, ta, tb) in enumerate(INSTR_PLAN):
        if kind == "hh":
            out[:, :, :, i, 0] = whT[:, :, :, ta]
            out[:, :, :, i, 1] = whT[:, :, :, tb]
        elif kind == "hhl":
            out[:, :, :, i, 0] = whT[:, :, :, 8]
            out[:, :, :, i, 1] = whT[:, :, :, 8]
        elif kind == "corr":
            out[:, :, :, i, 0] = wlT[:, :, :, ta]
            out[:, :, :, i, 1] = whT[:, :, :, ta]
        else:  # ll
            out[:, :, :, i, 0] = wlT[:, :, :, 8]
            out[:, :, :, i, 1] = wlT[:, :, :, 8]
    return np.ascontiguousarray(
        out.reshape(n_cot * 4 * 128, N_INSTR * 256))


def kernel(**inputs) -> np.ndarray:
    xt = np.ascontiguousarray(np.asarray(inputs["xt"], np.float32))
    xm = np.ascontiguousarray(np.asarray(inputs["xm"], np.float32))
    alpha = np.asarray(inputs["alpha"], np.float32).reshape(1)
    w0 = np.asarray(inputs["w0"], np.float32)
    w1 = np.asarray(inputs["w1"], np.float32)

    w0p = pack_weights(w0, 4)
    w1p = pack_weights(w1, 2)
    pat = build_patterns()

    common = {"alpha": alpha, "pat": pat, "w0p": w0p, "w1p": w1p}
    for nm in ("g0", "b0", "m0", "v0"):
        common[nm] = np.asarray(inputs[nm], np.float32).reshape(512, 1)
    for nm in ("g1", "b1", "m1", "v1"):
        common[nm] = np.asarray(inputs[nm], np.float32).reshape(256, 1)

    in_maps = []
    for b in range(N_CORES):
        m = dict(common)
        m["xt"] = np.ascontiguousarray(xt[b].reshape(512, 1024))
        m["xm"] = np.ascontiguousarray(xm[b].reshape(512, 4096))
        in_maps.append(m)

    nc = _get_nc()
    res = run_bass_kernel_spmd(nc, in_maps, core_ids=list(range(N_CORES)))
    out = np.stack([res.results[b]["out"].reshape(256, 64, 64)
                    for b in range(N_CORES)], axis=0)
    return out.astype(np.float32)
